# revision 25
# baseline (speedup 1.0000x reference)
"""Trainium2 Bass kernel for masked multi-head attention (nn_Attention_25271587569919).

Problem: B=4, S=2048, D=1024, 16 heads x 64. out = (softmax(QK^T/8 + pad/causal mask) V) WO.

Sharding: 8 cores = 4 batches x 2 head-groups (Megatron-style). Each core computes,
for its batch b and its 8 heads:
    QT/KT = (X Wq_g)^T in [dh, seq] layout,  V = X Wv_g in [seq, dh] layout,
    S^T tiles [k, q] (so pad mask = per-partition ACT bias, no transposes anywhere),
    P = exp(S^T/8 + pad) (no max subtraction; scores are O(1)),
    causal handled by narrowing the q-range of diagonal-straddling tiles plus one
    fused 128x(2x128) triangular 0/1 multiply on the diagonal block (post-exp,
    on the p tiles, deferred so it never head-of-line blocks the DVE),
    ctx^T (+rowsum via an all-ones 65th column of V) = Vaug^T @ P accumulated in PSUM,
    normalization: ctx evacuated to SBUF immediately (frees the PSUM banks),
    rowsum rows broadcast across 64 partitions by two adjacent K=1 matmuls into
    one shared PSUM slice, one fused reciprocal, two multiplies,
    then out_partial = ctx @ Wo_g.  Host sums the two head-group partials per batch.

Scheduling: a single statically-paced emission over all 160 (j,hp,i) attention
slots. The scores+exp stream runs AHEAD of the PV stream (p tiles buffered in a
deep SBUF pool), paced so the ScalarE exp queue always has backlog but the PE
never head-of-line blocks on an exp. Scores pairs are emitted two slots at a
time to halve PE tile-mode-switch drains. Projection/output-projection units are
spread between PV groups. Dummy warm-up matmuls run during the initial DMA so
the PE HAM clock-gate is at 8/8 before the first real matmul.
"""

import numpy as np
import ml_dtypes

BF = ml_dtypes.bfloat16
S = 2048
D = 1024
HG = 512          # head-group width (8 heads x 64)
DH = 64
NKT = 16          # seq tiles of 128 (k side)
NQT = 4           # seq tiles of 512 (q side)
NEG = -30000.0

LEAD_MAX = 14     # max scores-slots ahead of PV stream (ppool depth - 2)
SLACK_NS = 4500.0  # allowed ScalarE backlog beyond PE progress
N_WARMUP = 40

_CACHE = {}


def _build():
    import concourse.bass as bass  # noqa: F401
    import concourse.tile as tile
    from concourse import bacc, mybir

    f32 = mybir.dt.float32
    bf16 = mybir.dt.bfloat16
    Exp = mybir.ActivationFunctionType.Exp

    nc = bacc.Bacc("TRN2", target_bir_lowering=False, debug=False, num_devices=8)

    xq_d = nc.dram_tensor("xq", [128, NQT, 8, 512], bf16, kind="ExternalInput")
    xk_d = nc.dram_tensor("xk", [128, NQT, 8, 512], bf16, kind="ExternalInput")
    xv_d = nc.dram_tensor("xv", [128, NQT, 8, 512], bf16, kind="ExternalInput")
    wq_d = nc.dram_tensor("wq", [128, 8, 512], bf16, kind="ExternalInput")
    wk_d = nc.dram_tensor("wk", [128, 8, 512], bf16, kind="ExternalInput")
    wv_d = nc.dram_tensor("wv", [128, 8, 512], bf16, kind="ExternalInput")
    wo_d = nc.dram_tensor("wo", [128, 4, D], bf16, kind="ExternalInput")
    padb_d = nc.dram_tensor("padb", [128, NKT], f32, kind="ExternalInput")
    trim_d = nc.dram_tensor("trim", [128, 2, 128], bf16, kind="ExternalInput")
    out_d = nc.dram_tensor("out", [S, D], bf16, kind="ExternalOutput")

    # all 160 attention slots in processing order
    SLOTS = [(j, hp, i) for j in range(NQT) for hp in range(4) for i in range(4 * j + 4)]

    def lo(j, i):
        return max(0, (i - 4 * j) * 128)

    with tile.TileContext(nc) as tc:
        with (
            tc.tile_pool(name="consts", bufs=1) as consts,
            tc.tile_pool(name="big", bufs=1) as big,
            tc.tile_pool(name="xpool", bufs=4) as xpool,
            tc.tile_pool(name="ppool", bufs=LEAD_MAX + 2) as ppool,
            tc.tile_pool(name="cspool", bufs=4) as cspool,
            tc.tile_pool(name="rspool", bufs=3) as rspool,
            tc.tile_pool(name="tmppool", bufs=3) as tmppool,
            tc.tile_pool(name="outpool", bufs=2) as outpool,
            tc.tile_pool(name="pspool", bufs=3, space="PSUM") as pspool,
            tc.tile_pool(name="ctxpool", bufs=2, space="PSUM") as ctxpool,
        ):
            wq_sb = consts.tile([128, 8, HG], bf16, tag="wq")
            wk_sb = consts.tile([128, 8, HG], bf16, tag="wk")
            wv_sb = consts.tile([128, 8, HG], bf16, tag="wv")
            wo_sb = consts.tile([128, 4, D], bf16, tag="wo")
            padb_sb = consts.tile([128, NKT], f32, tag="padb")
            trim_sb = consts.tile([128, 2, 128], bf16, tag="trim")
            ones_sb = consts.tile([128, 128], bf16, tag="ones")

            qt_sb = big.tile([128, 4, S], bf16, tag="qt")    # (X Wq)^T : rows = dh
            kt_sb = big.tile([128, 4, S], bf16, tag="kt")
            vaug_sb = big.tile([128, NKT, 8 * 65], bf16, tag="vaug")  # V + ones col
            ctxt_sb = big.tile([128, 4, S], bf16, tag="ctxt")
            vaug_h = vaug_sb.rearrange("p m (h e) -> p m h e", e=65)

            nc.vector.memset(ones_sb, 1.0)
            nc.vector.memset(vaug_h[:, :, :, 64:65], 1.0)

            # ---- warm-up: keep PE busy during initial DMA so HAM goes 8/8 ----
            for w in range(N_WARMUP):
                # K=1 so only one PE-array row is active: warms the HAM clock
                # gate without the synchronized all-core power spike of a full
                # 128x128 all-ones matmul burst
                wp = pspool.tile([128, 2, 512], f32, tag="ps")
                nc.tensor.matmul(wp[:, 0, 0:128], lhsT=ones_sb[0:1, :],
                                 rhs=ones_sb[0:1, :], start=True, stop=True)

            # ---- initial loads (chunked so the first projection starts early) ----
            # weights for Q first (2 chunks), then per-kt xq stripe-0 chunks
            nc.sync.dma_start(out=wq_sb[:, 0:4, :], in_=wq_d.ap()[:, 0:4, :])
            nc.sync.dma_start(out=wq_sb[:, 4:8, :], in_=wq_d.ap()[:, 4:8, :])
            nc.sync.dma_start(out=padb_sb, in_=padb_d.ap())
            nc.sync.dma_start(out=trim_sb, in_=trim_d.ap())

            # ---------- stripe-A: projections for q/seq stripe j ----------
            def a_units(j, first=False):
                st = {}

                def load_x(name, dram, chunked=False):
                    def emit():
                        t = xpool.tile([128, 8, 512], bf16, tag="x")
                        if chunked:
                            for kt in range(8):
                                nc.sync.dma_start(out=t[:, kt, :],
                                                  in_=dram.ap()[:, j, kt, :])
                        else:
                            nc.sync.dma_start(out=t, in_=dram.ap()[:, j, :, :])
                        st[name] = t
                    return emit

                def load_w(dst, dram):
                    def emit():
                        nc.sync.dma_start(out=dst, in_=dram.ap())
                    return emit

                def proj_t(w_sb, dst_sb, t, x_name):
                    def emit():
                        ps = pspool.tile([128, 2, 512], f32, tag="ps")
                        for kt in range(8):
                            nc.tensor.matmul(
                                ps[:, 0, :],
                                lhsT=w_sb[:, kt, 128 * t:128 * (t + 1)],
                                rhs=st[x_name][:, kt, :],
                                start=(kt == 0), stop=(kt == 7),
                            )
                        nc.vector.tensor_copy(
                            out=dst_sb[:, t, 512 * j:512 * (j + 1)], in_=ps[:, 0, :])
                    return emit

                def proj_v(m):
                    def emit():
                        ps = pspool.tile([128, 2, 512], f32, tag="ps")
                        for kt in range(8):
                            nc.tensor.matmul(
                                ps[:, 0, :],
                                lhsT=st["xv"][:, kt, 128 * (m - 4 * j):128 * (m - 4 * j + 1)],
                                rhs=wv_sb[:, kt, :],
                                start=(kt == 0), stop=(kt == 7),
                            )
                        nc.vector.tensor_copy(
                            out=vaug_h[:, m, :, 0:64],
                            in_=ps[:, 0, :].rearrange("p (h e) -> p h e", e=64),
                        )
                    return emit

                if first:
                    # latency-critical: interleave loads with their consumers
                    # so the first projection starts as early as possible
                    units = [(0.0, load_x("xq", xq_d)),
                             (0.0, load_w(wk_sb, wk_d))]
                    for t in range(4):
                        units.append((1800.0, proj_t(wq_sb, qt_sb, t, "xq")))
                    units.append((0.0, load_x("xk", xk_d)))
                    units.append((0.0, load_w(wv_sb, wv_d)))
                    for t in range(4):
                        units.append((1800.0, proj_t(wk_sb, kt_sb, t, "xk")))
                    units.append((0.0, load_x("xv", xv_d)))
                    units.append((0.0, load_w(wo_sb, wo_d)))
                    for m in range(4 * j, 4 * j + 4):
                        units.append((1800.0, proj_v(m)))
                    return units
                # steady state: issue all three stripe loads up front so the
                # transfers complete long before the projection units (and the
                # scores reading their outputs) need them
                units = [(0.0, load_x("xq", xq_d)),
                         (0.0, load_x("xk", xk_d)),
                         (0.0, load_x("xv", xv_d))]
                for t in range(4):
                    units.append((1800.0, proj_t(wq_sb, qt_sb, t, "xq")))
                for t in range(4):
                    units.append((1800.0, proj_t(wk_sb, kt_sb, t, "xk")))
                for m in range(4 * j, 4 * j + 4):
                    units.append((1800.0, proj_v(m)))
                return units

            # ---------- stripe-C: output projection for q stripe j ----------
            def c_units(j):
                holder = {}

                def wo_mn(m, n):
                    def emit():
                        if n == 0:
                            o_new = outpool.tile([128, D], bf16, tag="o")
                            holder[m] = o_new
                        o = holder[m]
                        ps = pspool.tile([128, 2, 512], f32, tag="ps")
                        for kt in range(4):
                            nc.tensor.matmul(
                                ps[:, 0, :],
                                lhsT=ctxt_sb[:, kt, 128 * m:128 * (m + 1)],
                                rhs=wo_sb[:, kt, 512 * n:512 * (n + 1)],
                                start=(kt == 0), stop=(kt == 3),
                            )
                        nc.vector.tensor_copy(out=o[:, 512 * n:512 * (n + 1)],
                                              in_=ps[:, 0, :])
                        if n == 1:
                            nc.gpsimd.dma_start(
                                out=out_d.ap()[128 * m:128 * (m + 1), :], in_=o)
                    return emit
                return [(930.0, wo_mn(m, n))
                        for m in range(4 * j, 4 * j + 4) for n in range(2)]

            # ---------- the global attention emission ----------
            # state for the scores/exp stream (cursor m) and PV stream (cursor k)
            p_tiles = {}    # slot -> p sbuf tile
            ctx_tiles = {}  # (j,hp) -> (ctx_a, ctx_b)
            pending_norm = []
            pending_trim = {}

            def emit_scores_exp(mslot):
                j, hp, i = SLOTS[mslot]
                c = lo(j, i)
                ks = slice(128 * i, 128 * (i + 1))
                sp = pspool.tile([128, 2, 512], f32, tag="ps")
                nc.tensor.matmul(
                    sp[:, 0, c:], lhsT=kt_sb[0:64, hp, ks],
                    rhs=qt_sb[0:64, hp, 512 * j + c:512 * (j + 1)],
                    start=True, stop=True)
                nc.tensor.matmul(
                    sp[:, 1, c:], lhsT=kt_sb[64:128, hp, ks],
                    rhs=qt_sb[64:128, hp, 512 * j + c:512 * (j + 1)],
                    start=True, stop=True)
                p = ppool.tile([128, 2, 512], bf16, tag="p")
                nc.scalar.activation(
                    out=p[:, :, c:], in_=sp[:, :, c:], func=Exp,
                    bias=padb_sb[:, i:i + 1], scale=0.125)
                if i >= 4 * j:
                    # diagonal-straddling tile: fused triangular mask multiply,
                    # deferred so it is emitted once its exp has surely finished
                    # (avoids DVE head-of-line blocking behind a far-future exp)
                    pending_trim[mslot] = (p, c)
                p_tiles[mslot] = p

            def flush_trims(upto):
                for s in sorted(pending_trim):
                    if s > upto:
                        break
                    p, c = pending_trim.pop(s)
                    nc.vector.tensor_mul(
                        p[:, :, c:c + 128], p[:, :, c:c + 128], trim_sb)

            def emit_pv(kslot):
                j, hp, i = SLOTS[kslot]
                c = lo(j, i)
                ni = 4 * j + 4
                if i == 0:
                    ctx_tiles[(j, hp)] = (
                        ctxpool.tile([65, 512], f32, tag="ctx", name="ctx_a"),
                        ctxpool.tile([65, 512], f32, tag="ctx", name="ctx_b"),
                    )
                ctx_a, ctx_b = ctx_tiles[(j, hp)]
                flush_trims(kslot)  # safety: trim(k) must precede PV(k)
                p = p_tiles.pop(kslot)
                h0, h1 = 2 * hp, 2 * hp + 1
                nc.tensor.matmul(
                    ctx_a[:, c:], lhsT=vaug_sb[:, i, 65 * h0:65 * h0 + 65],
                    rhs=p[:, 0, c:],
                    start=(i == 0), stop=(i == ni - 1))
                nc.tensor.matmul(
                    ctx_b[:, c:], lhsT=vaug_sb[:, i, 65 * h1:65 * h1 + 65],
                    rhs=p[:, 1, c:],
                    start=(i == 0), stop=(i == ni - 1))
                return ctx_a, ctx_b

            def emit_norm_start(j, hp):
                """Evacuate ctx+rowsum to SBUF right away (frees the ctx PSUM
                banks fast). The finisher broadcasts the rowsum rows across 64
                partitions with two K=1 matmuls into one shared PSUM slice, a
                single fused reciprocal, then the two normalization multiplies."""
                ctx_a, ctx_b = ctx_tiles[(j, hp)]
                qs = slice(512 * j, 512 * (j + 1))
                cs_a = cspool.tile([65, 512], bf16, tag="cs", name="cs_a")
                cs_b = cspool.tile([65, 512], bf16, tag="cs", name="cs_b")
                nc.vector.tensor_copy(out=cs_a, in_=ctx_a)
                nc.vector.tensor_copy(out=cs_b, in_=ctx_b)
                del ctx_tiles[(j, hp)]

                def finish():
                    bc = pspool.tile([128, 2, 512], f32, tag="ps", name="bc")
                    nc.tensor.matmul(bc[0:64, 0, :], lhsT=ones_sb[64:65, 0:64],
                                     rhs=cs_a[64:65, :], start=True, stop=True)
                    nc.tensor.matmul(bc[0:64, 1, :], lhsT=ones_sb[64:65, 0:64],
                                     rhs=cs_b[64:65, :], start=True, stop=True)
                    rb = rspool.tile([64, 2, 512], f32, tag="rb", name="rb")
                    nc.vector.reciprocal_approx_fast(rb, bc[0:64, :, :])
                    nc.vector.tensor_mul(
                        ctxt_sb[0:64, hp, qs], cs_a[0:64, :], rb[:, 0, :])
                    tmp = tmppool.tile([64, 512], bf16, tag="tmp", name="tmp")
                    nc.vector.tensor_mul(tmp, cs_b[0:64, :], rb[:, 1, :])
                    nc.gpsimd.dma_start(out=ctxt_sb[64:128, hp, qs], in_=tmp)
                return finish

            # cost model (ns) for pacing
            def exp_cost(mslot):
                j, hp, i = SLOTS[mslot]
                return (2 * (512 - lo(j, i)) + 352) / 1.2

            def sc_cost(mslot):
                j, hp, i = SLOTS[mslot]
                return (512 - lo(j, i)) / 2.4 + 30.0

            def pv_cost(kslot):
                j, hp, i = SLOTS[kslot]
                return 2 * (512 - lo(j, i)) / 2.4 + 20.0

            # ---- emit a(0) fully (it is the kernel head; DMA-paced) ----
            for cost, u in a_units(0, first=True):
                u()

            # per-stripe unit queues
            stripe_units = {
                0: a_units(1),
                1: a_units(2) + c_units(0),
                2: a_units(3) + c_units(1),
                3: c_units(2),
            }

            t_pe = 0.0
            t_act = 0.0
            m = 0  # scores cursor
            units_emitted = {j: 0 for j in range(NQT)}

            slot_of_stripe = {}
            for idx, (j, hp, i) in enumerate(SLOTS):
                slot_of_stripe.setdefault(j, []).append(idx)

            def scores_ready(mslot):
                """qt/kt for this slot's (stripe, hp) must have been EMITTED
                already, else the read gets no dependency on the write."""
                jm, hpm, im = SLOTS[mslot]
                if jm == 0:
                    return True
                # in stripe_units[jm-1] = a(jm) [+ c units], proj_k(t=hpm) is at
                # index 7 + hpm ([load_xq, load_xk, load_xv, pq0..3, pk0..3, ...])
                return units_emitted[jm - 1] >= 8 + hpm

            for k in range(len(SLOTS)):
                j, hp, i = SLOTS[k]
                ni = 4 * j + 4

                # flush the previous group's norm finisher before this group's
                # first PV (its ctx psum buffer reuse waits on those multiplies)
                if pending_norm and i == 0:
                    pending_norm.pop(0)()
                # flush trims whose exp has had time to complete
                flush_trims(min(k + 2, m - 1))

                # scores stream: emit two slots at a time on alternating PV
                # slots (halves the PE tile-mode switch count vs one pair per
                # slot; the lead cap otherwise degenerates emission to singles)
                attempt = (k % 2 == 0) or (m - k) < 4 or m <= k + 1
                while attempt and m < len(SLOTS) and (m - k) < LEAD_MAX \
                        and scores_ready(m):
                    take = 1
                    if (m + 1 < len(SLOTS) and (m + 1 - k) < LEAD_MAX
                            and scores_ready(m + 1)):
                        take = 2
                    need = sum(exp_cost(m + t) for t in range(take))
                    if t_act + need > t_pe + SLACK_NS and m > k:
                        if take == 2 and t_act + exp_cost(m) <= t_pe + SLACK_NS:
                            take = 1  # pair does not fit the backlog; single does
                        else:
                            break
                    for t in range(take):
                        emit_scores_exp(m)
                        t_act += exp_cost(m)
                        t_pe += sc_cost(m)
                        m += 1

                # PV for slot k
                emit_pv(k)
                t_pe += pv_cost(k)

                if i == ni - 1:
                    pending_norm.append(emit_norm_start(j, hp))

                # interleaved units, spread over the stripe by slot share
                units = stripe_units.get(j)
                if units:
                    sl = slot_of_stripe[j]
                    done = sl.index(k) + 1
                    want = (done * len(units) + len(sl) - 1) // len(sl)
                    while units_emitted[j] < min(want, len(units)):
                        cost, u = units[units_emitted[j]]
                        u()
                        t_pe += cost
                        units_emitted[j] += 1

            # drain leftovers
            for j in range(NQT):
                units = stripe_units.get(j, [])
                while units_emitted[j] < len(units):
                    units[units_emitted[j]][1]()
                    units_emitted[j] += 1
            while pending_norm:
                pending_norm.pop(0)()
            for cost, u in c_units(NQT - 1):
                u()

    nc.compile()
    return nc


def _make_trim():
    p = np.arange(128)[:, None]
    f = np.arange(128)[None, :]
    tri = (f >= p).astype(np.float32).astype(BF)
    return np.stack([tri, tri], axis=1).copy()  # [128, 2, 128]


def _x_layout(X):
    # X: [S, D] f32 -> [128, NQT, 8, 512] bf16, per-partition-contiguous stripes
    return np.ascontiguousarray(
        X.T.reshape(8, 128, NQT, 512).transpose(1, 2, 0, 3)).astype(BF)


def _w_layout(W):
    # W: [D, HG] -> [128, 8, 512]
    return np.ascontiguousarray(W.reshape(8, 128, HG).transpose(1, 0, 2)).astype(BF)


def _wo_layout(W):
    # W: [HG, D] -> [128, 4, D]
    return np.ascontiguousarray(W.reshape(4, 128, D).transpose(1, 0, 2)).astype(BF)


def kernel(Q_emb, K_emb, V_emb, Q_ini, K_ini, WQ, WK, WV, WO):
    from concourse.bass_utils import run_bass_kernel_spmd

    if "nc" not in _CACHE:
        _CACHE["nc"] = _build()
    nc = _CACHE["nc"]

    Q_emb = np.asarray(Q_emb, np.float32)
    K_emb = np.asarray(K_emb, np.float32)
    V_emb = np.asarray(V_emb, np.float32)
    K_ini = np.asarray(K_ini)
    WQ = np.asarray(WQ, np.float32)
    WK = np.asarray(WK, np.float32)
    WV = np.asarray(WV, np.float32)
    WO = np.asarray(WO, np.float32)

    trim = _make_trim()
    in_maps = []
    for c in range(8):
        b, g = c // 2, c % 2
        gs = slice(HG * g, HG * (g + 1))
        padb = np.where(K_ini[b] != 0, 0.0, NEG).astype(np.float32)
        if padb[0] != 0.0:
            # key 0 masked would make causal row 0 fully masked -> rowsum 0 ->
            # NaN. The reference emits an (arbitrary) softmax over masked
            # scores there; keep key 0 live so output stays finite.
            padb[0] = 0.0
        in_maps.append({
            "xq": _x_layout(Q_emb[b]),
            "xk": _x_layout(K_emb[b]),
            "xv": _x_layout(V_emb[b]),
            "wq": _w_layout(WQ[:, gs]),
            "wk": _w_layout(WK[:, gs]),
            "wv": _w_layout(WV[:, gs]),
            "wo": _wo_layout(WO[gs, :]),
            "padb": padb.reshape(NKT, 128).T.copy(),
            "trim": trim,
        })

    _CACHE["in_maps"] = in_maps
    res = run_bass_kernel_spmd(nc, in_maps, list(range(8)))
    parts = [res.results[c]["out"].astype(np.float32) for c in range(8)]
    out = np.stack([parts[2 * b] + parts[2 * b + 1] for b in range(4)])
    return out.astype(np.float32)
